# revision 8
# baseline (speedup 1.0000x reference)
"""Distributed Trainium2 Bass kernel for nn_Attention_57346403336225.

Reference computation (per batch b of 16, N=1024 tokens, E=128 emb, H=8 heads,
head dim d = E = 128, INNER = 1024):
    xn   = LayerNorm(x) * ln_w + ln_b
    qkv  = xn @ w_qkv ; q,k,v heads
    dots = (q @ k^T) * scale[h]  ; diagonal masked to -FLT_MAX
    attn = softmax(dots) ; out = attn @ v
    y    = out @ w_proj + b_proj

Sharding: pure data-parallel over batch (16 / 8 cores = 2 batches per core),
weights replicated, no collectives.

Per-core algorithm (all heads/batches looped on-chip):
  - host precomputes A_h = scale[h] * Wq_h @ Wk_h^T  [E,E]  so that
    dots_h = xn @ A_h @ xn^T  (one [E,E] matmul replaces separate q,k)
  - LayerNorm in [tok, E] layout (bn_stats), PE-transpose -> xnT [E, tok];
    prologue emits the batch-0 chain stage-major (all stats, all aggrs, ...)
    so each engine queue holds runs of independent ops; the v-projection
    matmuls/casts trail into the first attention iterations.
  - dots tiles [128q, 1024k] in PSUM (bf16 matmuls), diagonal mask added on
    DVE, exp on ScalarE with fused per-row accumulation (softmax denom),
    P stored bf16
  - P^T via DMA xbar transposes batched 4 query-tiles per call (the xbar
    launch overhead dominates small calls); output in a strip layout
    [k 128, strip 32, q 128] consumed by PV with a strided AP
  - out^T accumulation over k chunks (bf16); projection per head with the
    softmax normalization (1/rowsum) applied as a per-partition scale in
    the projection epilogue, accumulated over heads on DVE. proj is emitted
    with pushed-back scheduler priority so its small matmuls don't block
    the next iteration's dots matmuls at the iteration boundary.
  - Sync engine owns only the xbar transposes; x loads / output stores go
    through the gpsimd SWDGE queue; weight loads on gpsimd in the prologue.
"""

import numpy as np
import ml_dtypes

B, N, E, H = 16, 1024, 128, 8
NCORES = 8
B_LOC = B // NCORES  # 2
LN_EPS = 1e-5
NT = N // 128    # 8 token tiles per batch
MASK_VAL = -1e30

_cache = {}


def _build_nc():
    import concourse.bacc as bacc
    import concourse.mybir as mybir
    import concourse.tile as tile

    f32 = mybir.dt.float32
    bf16 = mybir.dt.bfloat16
    Exp = mybir.ActivationFunctionType.Exp
    Sqrt = mybir.ActivationFunctionType.Sqrt
    sub = mybir.AluOpType.subtract
    mult = mybir.AluOpType.mult
    add = mybir.AluOpType.add

    nc = bacc.Bacc("TRN2", target_bir_lowering=False)

    x_p = nc.declare_dram_parameter("x", [B_LOC, N, E], f32, isOutput=False)
    a_p = nc.declare_dram_parameter("amat", [H, E, E], bf16, isOutput=False)
    wvf_p = nc.declare_dram_parameter("wvf", [E, H * E], bf16, isOutput=False)
    wp_p = nc.declare_dram_parameter("wp", [H, E, E], bf16, isOutput=False)
    lnw_p = nc.declare_dram_parameter("lnw", [E, 1], f32, isOutput=False)
    lnb_p = nc.declare_dram_parameter("lnb", [E, 1], f32, isOutput=False)
    id_p = nc.declare_dram_parameter("ident", [E, E], f32, isOutput=False)
    dm_p = nc.declare_dram_parameter("dmask", [E, E], f32, isOutput=False)
    bp_p = nc.declare_dram_parameter("bptile", [128, E], f32, isOutput=False)
    out_p = nc.declare_dram_parameter("out", [B_LOC, N, E], f32, isOutput=True)

    with tile.TileContext(nc) as tc:
        with (
            tc.tile_pool(name="const", bufs=1) as cpool,
            tc.tile_pool(name="ln", bufs=8) as lnpool,
            tc.tile_pool(name="work", bufs=4) as wpool,
            tc.tile_pool(name="bigP", bufs=2) as ppool,
            tc.tile_pool(name="bigPT", bufs=4) as ptpool,
            tc.tile_pool(name="psd", bufs=2, space="PSUM") as psd,
            tc.tile_pool(name="psm", bufs=2, space="PSUM") as psm,
            tc.tile_pool(name="psy", bufs=2, space="PSUM") as psy,
        ):
            # ---- constants ----
            ident = cpool.tile([E, E], f32, tag="ident")
            dmask = cpool.tile([E, E], f32, tag="dmask")
            lnw = cpool.tile([E, 1], f32, tag="lnw")
            lnb = cpool.tile([E, 1], f32, tag="lnb")
            amat = cpool.tile([E, H, E], bf16, tag="amat")
            wvf = cpool.tile([E, H * E], bf16, tag="wvf")
            wp = cpool.tile([E, H, E], bf16, tag="wp")
            bptile = cpool.tile([128, E], f32, tag="bptile")
            epst = cpool.tile([128, 1], f32, tag="epst")
            zbias = cpool.tile([128, 1], f32, tag="zbias")
            xall = [cpool.tile([128, NT, E], f32, tag=f"xall{b}",
                               name=f"xall{b}") for b in range(B_LOC)]
            xnT = [cpool.tile([E, N], bf16, tag=f"xnT{b}", name=f"xnT{b}")
                   for b in range(B_LOC)]
            vall = [cpool.tile([128, NT, H * E], bf16, tag=f"vall{b}",
                               name=f"vall{b}") for b in range(B_LOC)]
            y_acc = [cpool.tile([128, N], f32, tag=f"yacc{b}", name=f"yacc{b}")
                     for b in range(B_LOC)]

            # batch-0 x tiles first so LayerNorm can start right away
            for t in range(NT):
                nc.sync.dma_start(xall[0][:, t, :],
                                  x_p[0, t * 128:(t + 1) * 128, :])
            nc.sync.dma_start(ident[:], id_p[:])
            nc.sync.dma_start(lnw[:], lnw_p[:])
            nc.sync.dma_start(lnb[:], lnb_p[:])
            nc.gpsimd.dma_start(amat[:], a_p[:].rearrange("h a b -> a h b"))
            nc.gpsimd.dma_start(wvf[:], wvf_p[:])
            nc.gpsimd.dma_start(wp[:], wp_p[:].rearrange("h a b -> a h b"))
            nc.gpsimd.dma_start(dmask[:], dm_p[:])
            nc.gpsimd.dma_start(bptile[:], bp_p[:])
            for t in range(NT):
                nc.gpsimd.dma_start(xall[1][:, t, :],
                                    x_p[1, t * 128:(t + 1) * 128, :])
            nc.vector.memset(epst[:], LN_EPS)
            nc.vector.memset(zbias[:], 0.0)

            def ln_chain(b, ts_list):
                """LayerNorm for token tiles ts_list of batch b into xnT,
                emitted stage-major (independent runs per engine queue)."""
                tl = {}
                for t in ts_list:
                    st = lnpool.tile([128, 6], f32, tag="st",
                                     name=f"st{b}_{t}")
                    nc.vector.bn_stats(st[:], xall[b][:, t, :])
                    tl[t] = st
                mvl = {}
                for t in ts_list:
                    mv = lnpool.tile([128, 2], f32, tag="mv",
                                     name=f"mv{b}_{t}")
                    nc.vector.bn_aggr(mv[:], tl[t][:])
                    mvl[t] = mv
                sdl = {}
                for t in ts_list:
                    sd = lnpool.tile([128, 1], f32, tag="sd",
                                     name=f"sd{b}_{t}")
                    nc.scalar.activation(sd[:], mvl[t][:, 1:2], Sqrt,
                                         bias=epst[:])
                    sdl[t] = sd
                rsl = {}
                for t in ts_list:
                    rs = lnpool.tile([128, 1], f32, tag="rs",
                                     name=f"rs{b}_{t}")
                    nc.vector.reciprocal(rs[:], sdl[t][:])
                    rsl[t] = rs
                xnl = {}
                for t in ts_list:
                    xn = lnpool.tile([128, E], f32, tag="xnt",
                                     name=f"xn{b}_{t}")
                    nc.vector.tensor_scalar(
                        xn[:], xall[b][:, t, :], mvl[t][:, 0:1], rsl[t][:],
                        op0=sub, op1=mult,
                    )
                    xnl[t] = xn
                for t in ts_list:
                    tp = psm.tile([128, E], f32, tag="m512",
                                  name=f"lntp{b}_{t}")
                    nc.tensor.transpose(tp[:], xnl[t][:], ident[:])
                    nc.vector.tensor_scalar(
                        xnT[b][:, t * 128:(t + 1) * 128], tp[:],
                        lnw[:], lnb[:], op0=mult, op1=add,
                    )

            def v_proj(b, ts_list):
                """v = xn @ Wv for token tiles (all heads); casts on DVE."""
                for t in ts_list:
                    for c in range(2):
                        vps = psm.tile([128, 512], f32, tag="m512",
                                       name=f"vps{b}_{t}_{c}")
                        nc.tensor.matmul(
                            vps[:],
                            xnT[b][:, t * 128:(t + 1) * 128],
                            wvf[:, c * 512:(c + 1) * 512],
                            start=True, stop=True,
                        )
                        nc.vector.tensor_copy(
                            vall[b][:, t, c * 512:(c + 1) * 512], vps[:]
                        )

            # ---- attention, software-pipelined across (batch, head) ----
            iters = [(b, h) for b in range(B_LOC) for h in range(H)]
            NIT = len(iters)
            stash = {}

            def make_tT(it):
                b, h = iters[it]
                tT = wpool.tile([E, N], bf16, tag="tT", name=f"tT{it}")
                for qc in range(2):
                    tps = psm.tile([128, 512], f32, tag="m512",
                                   name=f"tps{it}_{qc}")
                    nc.tensor.matmul(
                        tps[:], amat[:, h, :],
                        xnT[b][:, qc * 512:(qc + 1) * 512],
                        start=True, stop=True,
                    )
                    nc.vector.tensor_copy(tT[:, qc * 512:(qc + 1) * 512],
                                          tps[:])
                stash[("tT", it)] = tT

            def dots_group(it, g, P, rsum):
                """Query tiles qt in [4g, 4g+4): dots matmuls, diag mask,
                exp with rowsum accumulation; then ONE xbar transpose of the
                4-tile block P[:, 4g:4g+4, :] ([128, 4096]) into the strip
                tile PTc[g]: PTc[p, s, q] = P^T for key (s % 8)*128 + p of
                query tile 4g + s//8."""
                b, h = iters[it]
                tT = stash[("tT", it)]
                for qt in range(4 * g, 4 * g + 4):
                    dps = psd.tile([128, N], f32, tag="dots",
                                   name=f"dps{it}_{qt}")
                    for kc in range(2):
                        nc.tensor.matmul(
                            dps[:, kc * 512:(kc + 1) * 512],
                            tT[:, qt * 128:(qt + 1) * 128],
                            xnT[b][:, kc * 512:(kc + 1) * 512],
                            start=True, stop=True,
                        )
                    nc.vector.tensor_add(
                        dps[:, qt * 128:(qt + 1) * 128],
                        dps[:, qt * 128:(qt + 1) * 128],
                        dmask[:],
                    )
                    nc.scalar.activation(
                        P[:, qt, :], dps[:], Exp, bias=zbias[:],
                        accum_out=rsum[:, qt:qt + 1],
                    )
                PTc = ptpool.tile([128, 4 * NT, 128], bf16, tag="PT",
                                  name=f"PTc{it}_{g}")
                nc.sync.dma_start(
                    out=PTc[:],
                    in_=P[:, 4 * g:4 * (g + 1), :],
                    transpose=True,
                )
                stash[("PT", it, g)] = PTc

            def pv_chain(it, qc):
                b, h = iters[it]
                PTc = stash[("PT", it, qc)]
                # strips for key tile kt are s = qt_local*8 + kt
                mov = PTc[:].rearrange("p (a s) q -> p a s q", s=NT)
                oT = stash[("oT", it)]
                ops = psm.tile([128, 512], f32, tag="m512",
                               name=f"ops{it}_{qc}")
                for kt in range(NT):
                    nc.tensor.matmul(
                        ops[:],
                        vall[b][:, kt, h * E:(h + 1) * E],
                        mov[:, :, kt, :],
                        start=(kt == 0), stop=(kt == NT - 1),
                    )
                nc.vector.tensor_copy(oT[:, qc * 512:(qc + 1) * 512], ops[:])

            def proj(it):
                b, h = iters[it]
                oT = stash.pop(("oT", it))
                rcp = stash.pop(("rcp", it))
                stash.pop(("tT", it))
                stash.pop(("PT", it, 0))
                stash.pop(("PT", it, 1))
                # pack 4 projection outputs per PSUM bank so the matmuls can
                # run ahead of the epilogue stt chain
                ypl = {}
                for grp in range(2):
                    yps4 = psy.tile([128, 512], f32, tag="yps",
                                    name=f"yps{it}_{grp}")
                    for j in range(4):
                        t = grp * 4 + j
                        nc.tensor.matmul(
                            yps4[:, j * 128:(j + 1) * 128],
                            oT[:, t * 128:(t + 1) * 128],
                            wp[:, h, :],
                            start=True, stop=True,
                        )
                        ypl[t] = yps4[:, j * 128:(j + 1) * 128]
                for t in range(NT):
                    if h == 0:
                        nc.vector.scalar_tensor_tensor(
                            y_acc[b][:, t * 128:(t + 1) * 128],
                            ypl[t], rcp[:, t:t + 1], bptile[:],
                            op0=mult, op1=add,
                        )
                    else:
                        nc.vector.scalar_tensor_tensor(
                            y_acc[b][:, t * 128:(t + 1) * 128],
                            ypl[t], rcp[:, t:t + 1],
                            y_acc[b][:, t * 128:(t + 1) * 128],
                            op0=mult, op1=add,
                        )
                    if h == H - 1 and t % 2 == 1:
                        nc.gpsimd.dma_start(
                            out_p[b, (t - 1) * 128:(t + 1) * 128].rearrange(
                                "(u p) e -> p u e", p=128),
                            y_acc[b][:, (t - 1) * 128:(t + 1) * 128].rearrange(
                                "p (u e) -> p u e", u=2),
                        )

            # batch-0 LayerNorm in the prologue (stage-major); v-projection
            # emitted after make_tT(0) so the first dots matmuls win the
            # tensor queue, v work drifts into iteration 0-1 slack.
            ln_chain(0, list(range(NT)))
            make_tT(0)
            with tc.high_priority(offset=-90):
                v_proj(0, list(range(NT)))

            # batch-1 LayerNorm tiles interleaved into the first iterations:
            # xnT[1] complete by end of it=6 (make_tT(8) is emitted at it=7),
            # vall[1] needed first at it=9 (pv of iter 8).
            b1_ln = {0: [0, 1], 1: [2], 2: [3], 3: [4], 4: [5], 5: [6],
                     6: [7]}

            for it in range(NIT + 2):
                cur = it if it < NIT else None
                if cur is not None:
                    b, h = iters[cur]
                    P = ppool.tile([128, NT, N], bf16, tag="P", name=f"P{cur}")
                    rsum = wpool.tile([128, NT], f32, tag="rsum",
                                      name=f"rsum{cur}")
                    oT = wpool.tile([E, N], bf16, tag="oT", name=f"oT{cur}")
                    stash[("oT", cur)] = oT
                    dots_group(cur, 0, P, rsum)
                if cur is not None and cur + 1 < NIT:
                    make_tT(cur + 1)
                if it - 1 >= 0 and it - 1 < NIT and it - 1 != NIT - 1:
                    pv_chain(it - 1, 0)
                if cur is not None:
                    dots_group(cur, 1, P, rsum)
                if it - 1 >= 0 and it - 1 < NIT:
                    pv_chain(it - 1, 1)
                if cur is not None:
                    rcp = wpool.tile([128, NT], f32, tag="rcp",
                                     name=f"rcp{cur}")
                    nc.vector.reciprocal(rcp[:], rsum[:])
                    stash[("rcp", cur)] = rcp
                for t in b1_ln.get(it, []):
                    ln_chain(1, [t])
                    v_proj(1, [t])
                if cur is not None and cur == NIT - 1:
                    # drain: start the last PV first-half early (its
                    # transpose lands mid-iteration)
                    pv_chain(cur, 0)
                if it - 2 >= 0 and it - 2 < NIT:
                    with tc.high_priority(offset=-60):
                        proj(it - 2)

    nc.compile()
    return nc


def _get_nc():
    if "nc" not in _cache:
        _cache["nc"] = _build_nc()
    return _cache["nc"]


def _make_in_maps(inputs):
    x = np.ascontiguousarray(np.asarray(inputs["x"], dtype=np.float32))
    ln_w = np.asarray(inputs["ln_w"], dtype=np.float32)
    ln_b = np.asarray(inputs["ln_b"], dtype=np.float32)
    w_qkv = np.asarray(inputs["w_qkv"], dtype=np.float32)
    scale = np.asarray(inputs["scale"], dtype=np.float32)
    w_proj = np.asarray(inputs["w_proj"], dtype=np.float32)

    INNER = E * H
    Wq = w_qkv[:, :INNER]
    Wk = w_qkv[:, INNER:2 * INNER]
    Wv = w_qkv[:, 2 * INNER:]

    amat = np.stack(
        [scale[h] * (Wq[:, h * E:(h + 1) * E] @ Wk[:, h * E:(h + 1) * E].T)
         for h in range(H)]
    ).astype(ml_dtypes.bfloat16)  # [H, E, E]
    wvf = Wv.astype(ml_dtypes.bfloat16)  # [E, INNER]
    wp = w_proj.reshape(H, E, E).astype(ml_dtypes.bfloat16)  # [H, d, E]
    ident = np.eye(E, dtype=np.float32)
    dmask = (np.eye(E, dtype=np.float32) * MASK_VAL).astype(np.float32)
    lnw = ln_w.reshape(E, 1)
    lnb = ln_b.reshape(E, 1)

    b_proj = np.asarray(inputs["b_proj"], dtype=np.float32)
    bptile = np.broadcast_to(b_proj[None, :], (128, E)).copy()
    shared = {
        "amat": amat, "wvf": wvf, "wp": wp, "bptile": bptile,
        "lnw": lnw, "lnb": lnb, "ident": ident, "dmask": dmask,
    }
    return [
        {"x": x[c * B_LOC:(c + 1) * B_LOC], **shared} for c in range(NCORES)
    ]


def kernel(x, ln_w, ln_b, w_qkv, scale, w_proj, b_proj):
    from concourse.bass_utils import run_bass_kernel_spmd

    in_maps = _make_in_maps(dict(
        x=x, ln_w=ln_w, ln_b=ln_b, w_qkv=w_qkv, scale=scale,
        w_proj=w_proj, b_proj=b_proj,
    ))

    nc = _get_nc()
    res = run_bass_kernel_spmd(nc, in_maps, core_ids=list(range(NCORES)))
    y = np.concatenate([res.results[c]["out"] for c in range(NCORES)], axis=0)
    return y.astype(np.float32)
